# revision 1
# baseline (speedup 1.0000x reference)
"""Trainium2 Bass kernel for nn_MultiHeadAttention (B=1, S=4096, d_model=768, 12 heads).

Sharding (8 cores): 2 head-groups (6 heads / 384 channels each) x 4 query-blocks
(1024 rows each).  Each core computes its head-group's contribution to the output
projection for its query block; the host sums the two head-group partials and adds
the constant bias terms.

Device algorithm per core (all layouts chosen so no on-device transposes needed):
  qT [384,1024]  = (Wq_g^T Qb^T) * s + bq_g * s        (s = 1/sqrt(768))
  kT [384,4096]  = Wk_g^T K^T                          (bk drops out of softmax)
  v  [4096,384]  = V Wv_g  (+ ones column per head)    (bv folded into host consts)
  scoresT[j,i]   = sum_d kT[d,j] qT[d,i]               (per head, psum)
  expT           = exp(scoresT)                        (ACT, no max subtraction:
                                                        |scores| <~ 1.5)
  pv[m,i]        = sum_j v_ext[j,m] expT[j,i]          (m<64: out_u^T, m=64: l)
  attn_T         = pv[0:64] * (1/l broadcast)
  OUT [1024,768] = attn_T^T @ Wo_g                     (partial, pre-bias)
Host: out[b] = OUT[g0,b] + OUT[g1,b] + (bv @ Wo + bo); plus exact bias algebra:
  bk contributes a per-row constant to scores -> softmax invariant -> dropped.
"""

import sys

sys.path.insert(0, "/opt/trn_rl_repo")

import numpy as np

import concourse.bass as bass
import concourse.mybir as mybir
from concourse.bass import ts, ds
from concourse.bass_utils import run_bass_kernel_spmd
from concourse.tile import TileContext

D_MODEL = 768
S = 4096
NH = 12
HD = 64
HG = 2                  # head groups
QB = 4                  # query blocks
C = D_MODEL // HG       # 384 channels per group
NHL = NH // HG          # 6 heads per group
QR = S // QB            # 1024 query rows per block
NCORES = 8
SCALE = float(1.0 / np.sqrt(np.float32(D_MODEL)))

F32 = mybir.dt.float32
F32R = mybir.dt.float32r
BF16 = mybir.dt.bfloat16
AF = mybir.ActivationFunctionType


def _r(ap):
    """View an fp32 AP as float32r for single-pass PE matmuls."""
    return ap.bitcast(F32R)


def _split_excess_waits(nc, max_waits=1):
    """walrus rejects instructions carrying more than one semaphore wait
    (setupSyncWait 'Too many sync wait commands').  Hoist excess waits onto
    no-op instructions inserted immediately before, on the same engine."""
    n_split = 0
    for f in nc.m.functions:
        for blk in f.blocks:
            new_insts = []
            for inst in blk.instructions:
                si = inst.sync_info
                if si is not None and si.on_wait and len(si.on_wait) > max_waits:
                    waits = list(si.on_wait)
                    keep = waits[-max_waits:]
                    extra = waits[:-max_waits]
                    for i in range(0, len(extra), max_waits):
                        chunk = extra[i : i + max_waits]
                        nop = mybir.InstNoOp(
                            name=f"{inst.name}_wsplit_{i}",
                            ins=[],
                            outs=[],
                            engine=inst.engine,
                            sync_info=mybir.SyncInfo(on_wait=chunk, on_update=[]),
                        )
                        new_insts.append(nop)
                        n_split += 1
                    si.on_wait = keep
                new_insts.append(inst)
            blk.instructions = new_insts
    return n_split


def _emit_body(nc, tc, io, use_f32r=True, stages=("proj", "attn", "oproj"), att_bf16=False,
               in_bf16=False, prof=None):
    QT, KT, VT, WQ, WK, WV, WO, BQ, OUT = (
        io["QT"], io["KT"], io["VT"], io["WQ"], io["WK"], io["WV"], io["WO"],
        io["BQ"], io["OUT"],
    )
    # float32r end-to-end: DRAM inputs are declared f32r, engine-produced
    # matmul operands are written as f32r (DVE/ACT round on write), so the
    # BIR verifier's "rounded to FP32r" rule is satisfied everywhere.
    DT = F32R if use_f32r else F32
    # attention-side dtype: bf16 halves nothing in cycle count but avoids the
    # slow f32r self-loading weight path and enables FWL on the PE
    DA = BF16 if att_bf16 else DT
    # input/projection-side dtype: bf16 halves the dominant input DMA traffic
    DI = BF16 if in_bf16 else DT

    consts = tc.alloc_tile_pool(name="consts", bufs=1)
    big = tc.alloc_tile_pool(name="big", bufs=1)

    # ---- weights -> SBUF ----
    wq_t, wk_t, wv_t = [], [], []
    for i in range(6):
        wq = consts.tile([128, C], DI, tag=f"wq{i}", name=f"wq{i}")
        d0 = nc.sync.dma_start(out=wq, in_=WQ[ts(i, 128), :])
        if prof is not None and i == 0:
            prof.snap(0, d0)
        wq_t.append(wq)
        wk = consts.tile([128, C], DI, tag=f"wk{i}", name=f"wk{i}")
        nc.sync.dma_start(out=wk, in_=WK[ts(i, 128), :])
        wk_t.append(wk)
        wv = consts.tile([128, C], DI, tag=f"wv{i}", name=f"wv{i}")
        nc.sync.dma_start(out=wv, in_=WV[ts(i, 128), :])
        wv_t.append(wv)
    wo_t = []
    for p in range(3):
        wo = consts.tile([128, D_MODEL], DT, tag=f"wo{p}", name=f"wo{p}")
        nc.sync.dma_start(out=wo, in_=WO[ts(p, 128), :])
        wo_t.append(wo)
    bq_t = []
    for p in range(3):
        bq = consts.tile([128, 1], F32, tag=f"bq{p}", name=f"bq{p}")
        nc.sync.dma_start(out=bq, in_=BQ[ts(p, 128)].rearrange("(p one) -> p one", one=1))
        bq_t.append(bq)
    ones64 = consts.tile([1, 64], F32, tag="ones64", name="ones64")
    nc.vector.memset(ones64, 1.0)
    # f32 ones source for the v_ext ones column (memset can't write f32r;
    # a DVE copy rounds f32 -> f32r)
    ones_col = consts.tile([128, NHL], F32, tag="ones_col", name="ones_col")
    nc.vector.memset(ones_col, 1.0)

    # ---- persistent activations ----
    qT_t = [big.tile([128, QR], DA, tag=f"qT{p}", name=f"qT{p}") for p in range(3)]
    kT_t = [big.tile([128, S], DA, tag=f"kT{p}", name=f"kT{p}") for p in range(3)]
    vext_t = [
        big.tile([128, NHL, HD + 1], DA, tag=f"vx{j}", name=f"vx{j}")
        for j in range(S // 128)
    ]

    # attention pools are allocated before the projection pools so the first
    # attention pair can interleave with vproj (LIFO release order).
    expp = tc.alloc_tile_pool(name="expp", bufs=2)
    attnp = tc.alloc_tile_pool(name="attnp", bufs=1)
    outp = tc.alloc_tile_pool(name="outp", bufs=1)
    smallp = tc.alloc_tile_pool(name="smallp", bufs=1)
    psS = tc.alloc_tile_pool(name="psS", bufs=2, space="PSUM")
    psV = tc.alloc_tile_pool(name="psV", bufs=2, space="PSUM")

    do_attn = "attn" in stages
    do_oproj = "oproj" in stages
    NKT = S // 128          # 32 key tiles
    GK = 2                  # key tiles per exp group

    def attn_pair_begin(qc, p):
        at = attnp.tile([128, 512], DT, tag=f"attn{qc}_{p}", name=f"attn{qc}_{p}")
        pvh = [
            psV.tile([HD + 1, 512], F32, tag="pv", name=f"pv{qc}_{p}_{h}")
            for h in range(2)
        ]
        return at, pvh

    def attn_group(qc, p, pvh, grp, offload_h1=True):
        es = []
        for h in range(2):
            sp = psS.tile([128, GK, 512], F32, tag="psS", name=f"sp{qc}_{p}_{grp}_{h}")
            for kt in range(GK):
                j = grp * GK + kt
                nc.tensor.matmul(
                    sp[:, kt, :],
                    lhsT=kT_t[p][ds(64 * h, 64), ts(j, 128)],
                    rhs=qT_t[p][ds(64 * h, 64), ts(qc, 512)],
                    start=True, stop=True,
                )
            e = expp.tile([128, GK, 512], DA, tag="exp", name=f"e{qc}_{p}_{grp}_{h}")
            if h == 0 or not offload_h1:
                # ACT reads PSUM at ~2.3 cyc/elem (vs 1.17 from SBUF); split
                # the softmax between ACT-direct and a DVE evacuation +
                # ACT-from-SBUF to balance the engines.
                nc.scalar.activation(e, sp, AF.Exp)
            else:
                s_sb = expp.tile([128, GK, 512], F32, tag="s_sb",
                                 name=f"ssb{qc}_{p}_{grp}_{h}", bufs=2)
                nc.vector.tensor_copy(s_sb, sp)
                nc.scalar.activation(e, s_sb, AF.Exp)
            es.append(e)
        for h in range(2):
            for kt in range(GK):
                j = grp * GK + kt
                nc.tensor.matmul(
                    pvh[h],
                    lhsT=vext_t[j][:, p * 2 + h, :],
                    rhs=es[h][:, kt, :],
                    start=(j == 0), stop=(j == NKT - 1),
                )

    def attn_pair_end(qc, p, at, pvh):
        for h in range(2):
            rr = smallp.tile([1, 512], F32, tag="rr", name=f"rr{qc}_{p}_{h}")
            nc.vector.reciprocal(rr, pvh[h][ds(HD, 1), :])
            rbc = psV.tile([64, 512], F32, tag="pv", name=f"rbc{qc}_{p}_{h}")
            nc.tensor.matmul(rbc, lhsT=ones64, rhs=rr, start=True, stop=True)
            pv_sb = smallp.tile([64, 512], F32, tag="pv_sb", name=f"pvsb{qc}_{p}_{h}")
            nc.vector.tensor_copy(pv_sb, pvh[h][ds(0, HD), :])
            gate = nc.vector.tensor_mul(at[ds(64 * h, 64), :], pv_sb, rbc)
            if prof is not None and h == 1:
                prof.snap(4 + qc * 3 + p, gate)
        return at

    # ================= phase A: projections =================
    instream = tc.alloc_tile_pool(name="instream", bufs=2)
    psA = tc.alloc_tile_pool(name="psA", bufs=2, space="PSUM")

    # kproj: kT = WK^T KT
    for ck in range(S // 512):
        kin = [instream.tile([128, 512], DI, tag=f"xin{i}", name=f"kin{i}_{ck}", bufs=2) for i in range(6)]
        for i in range(6):
            nc.sync.dma_start(out=kin[i], in_=KT[ts(i, 128), ts(ck, 512)])
        for p in range(3):
            ps = psA.tile([128, 512], F32, tag="psA", name=f"ps_k{p}_{ck}")
            for i in range(6):
                nc.tensor.matmul(
                    ps, lhsT=wk_t[i][:, ts(p, 128)], rhs=kin[i],
                    start=(i == 0), stop=(i == 5),
                )
            gate = nc.vector.tensor_copy(kT_t[p][:, ts(ck, 512)], ps)
            if prof is not None and ck == S // 512 - 1 and p == 2:
                prof.snap(2, gate)

    # qproj: qT = (WQ^T QT) * s + bq*s
    for qc in range(QR // 512):
        qin = [instream.tile([128, 512], DI, tag=f"xin{i}", name=f"qin{i}_{qc}", bufs=2) for i in range(6)]
        for i in range(6):
            nc.sync.dma_start(out=qin[i], in_=QT[ts(i, 128), ts(qc, 512)])
        for p in range(3):
            ps = psA.tile([128, 512], F32, tag="psA", name=f"ps_q{p}_{qc}")
            for i in range(6):
                nc.tensor.matmul(
                    ps, lhsT=wq_t[i][:, ts(p, 128)], rhs=qin[i],
                    start=(i == 0), stop=(i == 5),
                )
            gate = nc.scalar.activation(
                qT_t[p][:, ts(qc, 512)], ps, AF.Identity, bias=bq_t[p], scale=SCALE
            )
            if prof is not None and qc == QR // 512 - 1 and p == 2:
                prof.snap(1, gate)

    # vproj: v[key, ch] = sum_in VT[in, key] WV[in, ch], written per-head with a
    # ones column appended (lhsT for the pv matmul).  The first attention pair
    # (qc0, p0) is interleaved here: its scores/exp need only qT/kT (already
    # done) and its pv consumes v_ext key tiles right as vproj produces them,
    # so ACT/DVE softmax work hides under the DMA-bound vproj window.
    pair00 = attn_pair_begin(0, 0) if do_attn else None
    for ck in range(S // 512):
        vin = [instream.tile([128, 512], DI, tag=f"xin{i}", name=f"vin{i}_{ck}", bufs=2) for i in range(6)]
        for i in range(6):
            nc.sync.dma_start(out=vin[i], in_=VT[ts(i, 128), ts(ck, 512)])
        for ksub in range(4):
            j = ck * 4 + ksub
            ps = psA.tile([128, C], F32, tag="psA", name=f"ps_v{j}")
            for i in range(6):
                nc.tensor.matmul(
                    ps, lhsT=vin[i][:, ts(ksub, 128)], rhs=wv_t[i],
                    start=(i == 0), stop=(i == 5),
                )
            nc.vector.tensor_copy(
                vext_t[j][:, :, 0:HD], ps.rearrange("p (h d) -> p h d", h=NHL)
            )
            gate = nc.vector.tensor_copy(vext_t[j][:, :, HD], ones_col)
            if prof is not None and j == S // 128 - 1:
                prof.snap(3, gate)
        if do_attn:
            for grp in (2 * ck, 2 * ck + 1):
                attn_group(0, 0, pair00[1], grp)

    psA.release()
    instream.release()

    # ================= phase B: attention =================
    for qc in range(QR // 512):
        attn_tiles = []
        for p in range(3):
            if not do_attn:
                break
            if qc == 0 and p == 0:
                # already computed interleaved with vproj; just normalize
                at, pvh = pair00
            else:
                at, pvh = attn_pair_begin(qc, p)
                for grp in range(NKT // GK):
                    attn_group(qc, p, pvh, grp)
            attn_tiles.append(at)
            attn_pair_end(qc, p, at, pvh)
        # oproj for this q chunk: OUT[qc*512 + qs*128 .. , :] partial
        for qs in range(4):
            if not (do_attn and do_oproj):
                break
            ob = outp.tile([128, D_MODEL], F32, tag="ob", name=f"ob{qc}_{qs}")
            for oc in range(2):
                po = psV.tile([128, 384], F32, tag="pv", name=f"po{qc}_{qs}_{oc}")
                for p in range(3):
                    nc.tensor.matmul(
                        po,
                        lhsT=attn_tiles[p][:, ts(qs, 128)],
                        rhs=wo_t[p][:, ts(oc, 384)],
                        start=(p == 0), stop=(p == 2),
                    )
                nc.vector.tensor_copy(ob[:, ts(oc, 384)], po)
            gate = nc.sync.dma_start(out=OUT[ds(qc * 512 + qs * 128, 128), :], in_=ob)
            if prof is not None and qs == 3:
                prof.snap(10 + qc, gate)

    for pool in [psV, psS, smallp, outp, attnp, expp, big, consts]:
        pool.release()


_nc_cache = {}


PROF_LK = 256           # ladder length (ticks)
PROF_TICK_CYC = 4800    # NX cycles per tick  (~4us at 1.2 GHz)
PROF_NSNAP = 12


class _Prof:
    """On-device sampling profiler: a GPSIMD tick ladder (sequencer-only
    stores + fixed-cycle nops, invisible to Tile's dep tracker) plus snapshot
    DMAs of the tick buffer gated on phase-completion instructions."""

    def __init__(self, nc, prog_ap, PROG):
        self.nc = nc
        self.prog_ap = prog_ap
        self.PROG = PROG

    def snap(self, idx, gate):
        from concourse.tile_rust import add_dep_helper
        d = self.nc.sync.dma_start(out=self.PROG[ds(idx, 1), :], in_=self.prog_ap)
        add_dep_helper(d.ins, gate.ins, sync=True, reason=f"prof snap {idx}")


def _emit_prof_ladder(nc, prog_ap):
    """Emit (post-Tile) the Pool tick ladder, then relocate it to just after
    Pool's preamble-barrier instructions so it runs concurrently with the
    kernel body."""
    ladder = []
    reg_ctx = nc.gpsimd.register("prof_tick")
    reg = reg_ctx.__enter__()
    z = nc.gpsimd.reg_alu(reg, 0, 0, mybir.AluOpType.add)
    ladder.append(z.ins)
    for i in range(PROF_LK):
        s = nc.gpsimd.store(prog_ap[0:1, ds(i, 1)], reg)
        ladder.append(s.ins)
    for i in range(PROF_LK):
        a = nc.gpsimd.reg_alu(reg, reg, 1, mybir.AluOpType.add)
        ladder.append(a.ins)
        s = nc.gpsimd.store(prog_ap[0:1, ds(i, 1)], reg)
        ladder.append(s.ins)
        n = nc.gpsimd.nop(cycle_cnt=PROF_TICK_CYC, nofuse=True)
        ladder.append(n.ins)
    ladder_set = set(id(x) for x in ladder)
    f = nc.m.functions[0]
    # remove from wherever they were appended
    for blk in f.blocks:
        blk.instructions = [x for x in blk.instructions if id(x) not in ladder_set]
    # insert at the start of the TileContext body block so Pool runs the
    # ladder concurrently with the kernel (Pool is otherwise unused there)
    for blk in f.blocks:
        if blk.name.startswith("tile_context"):
            blk.instructions[0:0] = ladder
            return
    raise RuntimeError("profiler: no tile_context block found for tick ladder")


def build_nc(reps=1, use_f32r=True, split_waits=True, stages=("proj", "attn", "oproj"),
             timing_mode=False, att_bf16=False, in_bf16=False, profile_ladder=False):
    key = (reps, use_f32r, split_waits, tuple(stages), timing_mode, att_bf16, in_bf16,
           profile_ladder)
    if key in _nc_cache:
        return _nc_cache[key]
    nc = bass.Bass()
    DT = F32R if use_f32r else F32
    # attention-side dtype: bf16 halves nothing in cycle count but avoids the
    # slow f32r self-loading weight path and enables FWL on the PE
    DA = BF16 if att_bf16 else DT
    # input/projection-side dtype: bf16 halves the dominant input DMA traffic
    DI = BF16 if in_bf16 else DT
    if timing_mode:
        # timing-only variant: big tensors live in Internal DRAM so per-call
        # host->device staging is negligible; numerics are garbage.
        nc.declare_dram_parameter("DUMMY", [1, 128], F32, isOutput=False)
        io = {
            "QT": nc.dram_tensor("QT", [D_MODEL, QR], DI),
            "KT": nc.dram_tensor("KT", [D_MODEL, S], DI),
            "VT": nc.dram_tensor("VT", [D_MODEL, S], DI),
            "WQ": nc.dram_tensor("WQ", [D_MODEL, C], DI),
            "WK": nc.dram_tensor("WK", [D_MODEL, C], DI),
            "WV": nc.dram_tensor("WV", [D_MODEL, C], DI),
            "WO": nc.dram_tensor("WO", [C, D_MODEL], DT),
            "BQ": nc.dram_tensor("BQ", [C], F32),
            "OUT": nc.declare_dram_parameter("OUT", [QR, D_MODEL], F32, isOutput=True),
        }
    else:
        io = {
            "QT": nc.declare_dram_parameter("QT", [D_MODEL, QR], DI, isOutput=False),
            "KT": nc.declare_dram_parameter("KT", [D_MODEL, S], DI, isOutput=False),
            "VT": nc.declare_dram_parameter("VT", [D_MODEL, S], DI, isOutput=False),
            "WQ": nc.declare_dram_parameter("WQ", [D_MODEL, C], DI, isOutput=False),
            "WK": nc.declare_dram_parameter("WK", [D_MODEL, C], DI, isOutput=False),
            "WV": nc.declare_dram_parameter("WV", [D_MODEL, C], DI, isOutput=False),
            "WO": nc.declare_dram_parameter("WO", [C, D_MODEL], DT, isOutput=False),
            "BQ": nc.declare_dram_parameter("BQ", [C], F32, isOutput=False),
            "OUT": nc.declare_dram_parameter("OUT", [QR, D_MODEL], F32, isOutput=True),
        }
    prof = None
    prog_ap = None
    if profile_ladder:
        PROG = nc.declare_dram_parameter(
            "PROG", [PROF_NSNAP, PROF_LK], mybir.dt.int32, isOutput=True)
        prog_ap = nc.alloc_sbuf_tensor("prog_ticks", [1, PROF_LK], mybir.dt.int32).ap()
        prof = _Prof(nc, prog_ap, PROG)
    with TileContext(nc) as tc:
        for _ in range(reps):
            _emit_body(nc, tc, io, use_f32r=use_f32r, stages=stages, att_bf16=att_bf16,
                       in_bf16=in_bf16, prof=prof)
    if profile_ladder:
        _emit_prof_ladder(nc, prog_ap)
    if split_waits:
        _split_excess_waits(nc)
    _nc_cache[key] = nc
    return nc


def make_in_maps(Q, K, V, Wq, bq, Wk, bk, Wv, bv, Wo, bo, in_bf16=False):
    """Host-side sharding.  Returns (in_maps, host_const) where host_const is
    the [768] vector added to every output row (bv @ Wo + bo)."""
    Qm = np.asarray(Q, np.float32).reshape(S, D_MODEL)
    Km = np.asarray(K, np.float32).reshape(S, D_MODEL)
    Vm = np.asarray(V, np.float32).reshape(S, D_MODEL)
    QT = np.ascontiguousarray(Qm.T)
    KT = np.ascontiguousarray(Km.T)
    VT = np.ascontiguousarray(Vm.T)
    Wq = np.asarray(Wq, np.float32); Wk = np.asarray(Wk, np.float32)
    Wv = np.asarray(Wv, np.float32); Wo = np.asarray(Wo, np.float32)
    bq = np.asarray(bq, np.float32); bv = np.asarray(bv, np.float32)
    bo = np.asarray(bo, np.float32)

    import ml_dtypes
    def cvt(a):
        return np.ascontiguousarray(a).astype(ml_dtypes.bfloat16) if in_bf16 \
            else np.ascontiguousarray(a)
    in_maps = []
    for c in range(NCORES):
        g, b = divmod(c, QB)
        ch = slice(g * C, (g + 1) * C)
        in_maps.append({
            "QT": cvt(QT[:, b * QR : (b + 1) * QR]),
            "KT": cvt(KT),
            "VT": cvt(VT),
            "WQ": cvt(Wq[:, ch]),
            "WK": cvt(Wk[:, ch]),
            "WV": cvt(Wv[:, ch]),
            "WO": np.ascontiguousarray(Wo[ch, :]),
            # device computes qT = psum*SCALE + BQ, so prescale the bias here
            "BQ": np.ascontiguousarray(bq[ch] * np.float32(SCALE)),
        })
    host_const = (bv @ Wo + bo).astype(np.float32)
    return in_maps, host_const


def kernel(Q, K, V, Wq, bq, Wk, bk, Wv, bv, Wo, bo):
    nc = build_nc()
    in_maps, host_const = make_in_maps(Q, K, V, Wq, bq, Wk, bk, Wv, bv, Wo, bo)
    res = run_bass_kernel_spmd(nc, in_maps, core_ids=list(range(NCORES)))
    out = np.zeros((S, D_MODEL), np.float32)
    for c in range(NCORES):
        g, b = divmod(c, QB)
        out[b * QR : (b + 1) * QR, :] += res.results[c]["OUT"]
    out += host_const[None, :]
    return out.reshape(1, S, D_MODEL)



# revision 2
# speedup vs baseline: 1.2556x; 1.2556x over previous
"""Trainium2 Bass kernel v2 for nn_MultiHeadAttention (B=1, S=4096, d=768, 12 heads).

Sharding (8 cores): 2 head-groups (6 heads / 384 ch) x 4 query-blocks (1024 rows).
All-bf16 data path (f32 PSUM accum), f32r only for tiny broadcast matmuls.

Design (per core), driven by ACT (softmax exp) saturation:
  - pairs (qc, p) of 512 queries x 128 channels, p-major order:
    (0,0),(1,0),(0,1),(1,1),(0,2),(1,2)
  - per pair, 16 groups of GK=2 key-tiles: scores (4 MMs, row-group concurrent
    across the 2 heads) -> exp (ACT from PSUM, or DVE Schraudolph for selected
    groups) -> PV (8 MMs, contraction rg-split -> 2x concurrent; ones-row kept
    for the softmax denominator)
  - PSUM: psS h0/h1 slots (4 banks) shared by projections, scores and oproj;
    psV ring (4 banks) for PV accumulators + reciprocal-broadcast tiles.
  - projections: weights-resident, KT fully resident in SBUF, kproj p-major so
    pair (0,0) starts right after kproj p=0; vproj windows carry PV(0,0) and
    scores(1,0).
Host: sums the 2 head-group partials per query block and adds bv@Wo + bo.
"""

import sys

sys.path.insert(0, "/opt/trn_rl_repo")

import numpy as np

import concourse.bass as bass
import concourse.mybir as mybir
from concourse.bass import ts, ds
from concourse.bass_utils import run_bass_kernel_spmd
from concourse.tile import TileContext

D_MODEL = 768
S = 4096
NH = 12
HD = 64
HG = 2
QB = 4
C = D_MODEL // HG       # 384 channels per head-group
NHL = NH // HG          # 6 heads per group
QR = S // QB            # 1024 query rows per block
NCORES = 8
SCALE = float(1.0 / np.sqrt(np.float32(D_MODEL)))
NKT = S // 128          # 32 key tiles
GK = 2                  # key tiles per group
NG = NKT // GK          # 16 groups per pair

F32 = mybir.dt.float32
F32R = mybir.dt.float32r
BF16 = mybir.dt.bfloat16
I16 = mybir.dt.int16
AF = mybir.ActivationFunctionType
OP = mybir.AluOpType

# Schraudolph exp in bf16 bits: bits = s * (2^7/ln2) + SCHR_B  (i16, bitcast bf16)
SCHR_A = 184.6650390625
SCHR_B = 16250.4            # HW-calibrated: centers the sawtooth error at +-3.3%


def _r(ap):
    return ap.bitcast(F32R)


def _split_excess_waits(nc, max_waits=1):
    """walrus rejects instructions carrying more than one semaphore wait."""
    n_split = 0
    for f in nc.m.functions:
        for blk in f.blocks:
            new_insts = []
            for inst in blk.instructions:
                si = inst.sync_info
                if si is not None and si.on_wait and len(si.on_wait) > max_waits:
                    waits = list(si.on_wait)
                    keep = waits[-max_waits:]
                    extra = waits[:-max_waits]
                    for i in range(0, len(extra), max_waits):
                        chunk = extra[i : i + max_waits]
                        nop = mybir.InstNoOp(
                            name=f"{inst.name}_wsplit_{i}",
                            ins=[],
                            outs=[],
                            engine=inst.engine,
                            sync_info=mybir.SyncInfo(on_wait=chunk, on_update=[]),
                        )
                        new_insts.append(nop)
                        n_split += 1
                    si.on_wait = keep
                new_insts.append(inst)
            blk.instructions = new_insts
    return n_split


PROF_LK = 256
PROF_TICK_CYC = 4800
PROF_NSNAP = 12


class _Prof:
    def __init__(self, nc, prog_ap, PROG):
        self.nc = nc
        self.prog_ap = prog_ap
        self.PROG = PROG

    def snap(self, idx, gate):
        from concourse.tile_rust import add_dep_helper
        d = self.nc.sync.dma_start(out=self.PROG[ds(idx, 1), :], in_=self.prog_ap)
        add_dep_helper(d.ins, gate.ins, sync=True, reason=f"prof snap {idx}")


def _emit_prof_ladder(nc, prog_ap):
    ladder = []
    reg_ctx = nc.gpsimd.register("prof_tick")
    reg = reg_ctx.__enter__()
    z = nc.gpsimd.reg_alu(reg, 0, 0, OP.add)
    ladder.append(z.ins)
    for i in range(PROF_LK):
        s = nc.gpsimd.store(prog_ap[0:1, ds(i, 1)], reg)
        ladder.append(s.ins)
    for i in range(PROF_LK):
        a = nc.gpsimd.reg_alu(reg, reg, 1, OP.add)
        ladder.append(a.ins)
        s = nc.gpsimd.store(prog_ap[0:1, ds(i, 1)], reg)
        ladder.append(s.ins)
        n = nc.gpsimd.nop(cycle_cnt=PROF_TICK_CYC, nofuse=True)
        ladder.append(n.ins)
    ladder_set = set(id(x) for x in ladder)
    f = nc.m.functions[0]
    for blk in f.blocks:
        blk.instructions = [x for x in blk.instructions if id(x) not in ladder_set]
    for blk in f.blocks:
        if blk.name.startswith("tile_context"):
            blk.instructions[0:0] = ladder
            return
    raise RuntimeError("profiler: no tile_context block found for tick ladder")


def _emit_body(nc, tc, io, schr16=0, prof=None):
    QT, KT, VT, WQ, WK, WV, WO, BQ, OUT = (
        io["QT"], io["KT"], io["VT"], io["WQ"], io["WK"], io["WV"], io["WO"],
        io["BQ"], io["OUT"],
    )
    SCHR_GRPS = {
        0: set(), 1: {8}, 2: {5, 11}, 3: {4, 9, 14}, 4: {3, 7, 11, 15},
        5: {2, 5, 8, 11, 14}, 6: {1, 4, 7, 10, 13, 15}, 8: {1, 3, 5, 7, 9, 11, 13, 15},
    }[schr16]

    consts = tc.alloc_tile_pool(name="consts", bufs=1)
    persist = tc.alloc_tile_pool(name="persist", bufs=1)
    esp = tc.alloc_tile_pool(name="esp", bufs=6)
    attnp = tc.alloc_tile_pool(name="attnp", bufs=1)
    obp = tc.alloc_tile_pool(name="obp", bufs=2)
    smallp = tc.alloc_tile_pool(name="smallp", bufs=2)
    psS = tc.alloc_tile_pool(name="psS", bufs=1, space="PSUM")
    psV = tc.alloc_tile_pool(name="psV", bufs=4, space="PSUM")

    # ---- weights -> SBUF (bf16) ----
    wq_t, wk_t, wv_t = [], [], []
    for i in range(6):
        wq = consts.tile([128, C], BF16, tag=f"wq{i}", name=f"wq{i}")
        d0 = nc.sync.dma_start(out=wq, in_=WQ[ts(i, 128), :])
        if prof is not None and i == 0:
            prof.snap(0, d0)
        wq_t.append(wq)
    for i in range(6):
        wk = consts.tile([128, C], BF16, tag=f"wk{i}", name=f"wk{i}")
        nc.sync.dma_start(out=wk, in_=WK[ts(i, 128), :])
        wk_t.append(wk)
    for i in range(6):
        wv = consts.tile([128, C], BF16, tag=f"wv{i}", name=f"wv{i}")
        nc.sync.dma_start(out=wv, in_=WV[ts(i, 128), :])
        wv_t.append(wv)
    wo_t = []
    for p in range(3):
        wo = consts.tile([128, D_MODEL], BF16, tag=f"wo{p}", name=f"wo{p}")
        nc.sync.dma_start(out=wo, in_=WO[ts(p, 128), :])
        wo_t.append(wo)
    bq_t = []
    for p in range(3):
        bq = consts.tile([128, 1], F32, tag=f"bq{p}", name=f"bq{p}")
        nc.sync.dma_start(out=bq, in_=BQ[ts(p, 128)].rearrange("(p one) -> p one", one=1))
        bq_t.append(bq)
    ones64 = consts.tile([1, 64], F32, tag="ones64", name="ones64")
    nc.vector.memset(ones64, 1.0)
    ones_col = consts.tile([128, NHL], F32, tag="ones_col", name="ones_col")
    nc.vector.memset(ones_col, 1.0)

    # ---- persistent activations ----
    kT_t = [persist.tile([128, S], BF16, tag=f"kT{p}", name=f"kT{p}") for p in range(3)]
    qT_t = [persist.tile([128, QR], BF16, tag=f"qT{p}", name=f"qT{p}") for p in range(3)]
    vext_t = [persist.tile([128, NHL, HD + 1], BF16, tag=f"vx{j}", name=f"vx{j}")
              for j in range(NKT)]
    # inputs resident
    ktin = [persist.tile([128, S], BF16, tag=f"ktin{i}", name=f"ktin{i}") for i in range(6)]
    qin = [persist.tile([128, QR], BF16, tag=f"qin{i}", name=f"qin{i}") for i in range(6)]
    for i in range(6):
        nc.sync.dma_start(out=qin[i], in_=QT[ts(i, 128), :])
    for i in range(6):
        nc.sync.dma_start(out=ktin[i], in_=KT[ts(i, 128), :])
    vinp = tc.alloc_tile_pool(name="vinp", bufs=2)

    # psS slot allocator: alternate h0/h1 slots for projection chunks too
    slot_ctr = [0]

    def psS_slot():
        t = psS.tile([128, 2, 512], F32, tag=f"psS{slot_ctr[0] % 2}",
                     name=f"slot{slot_ctr[0]}")
        slot_ctr[0] += 1
        return t

    # ---------- projection units ----------
    def qproj_unit(p):
        sp = psS_slot()
        for half in range(2):
            for i in range(6):
                nc.tensor.matmul(
                    sp[:, half, :], lhsT=wq_t[i][:, ts(p, 128)],
                    rhs=qin[i][:, ds(half * 512, 512)],
                    start=(i == 0), stop=(i == 5),
                )
        return nc.vector.tensor_scalar(
            qT_t[p], sp.rearrange("p a b -> p (a b)"), SCALE, bq_t[p],
            OP.mult, OP.add)

    def kproj_unit(p, ck):
        sp = psS_slot()
        for half in range(2):
            for i in range(6):
                nc.tensor.matmul(
                    sp[:, half, :], lhsT=wk_t[i][:, ts(p, 128)],
                    rhs=ktin[i][:, ds(ck * 1024 + half * 512, 512)],
                    start=(i == 0), stop=(i == 5),
                )
        return nc.vector.tensor_copy(
            kT_t[p][:, ds(ck * 1024, 1024)], sp.rearrange("p a b -> p (a b)"))

    def vproj_slot(vin, ck, sl):
        # two key tiles j = ck*8 + sl*2 + {0,1}
        sp = psS_slot()
        gates = []
        for jj in range(2):
            j = ck * 8 + sl * 2 + jj
            for i in range(6):
                nc.tensor.matmul(
                    sp[:, jj, 0:C], lhsT=vin[i][:, ds((sl * 2 + jj) * 128, 128)],
                    rhs=wv_t[i], start=(i == 0), stop=(i == 5),
                )
            nc.vector.tensor_copy(
                vext_t[j][:, :, 0:HD],
                sp[:, jj, 0:C].rearrange("p (h d) -> p h d", h=NHL))
            g = nc.vector.tensor_copy(vext_t[j][:, :, HD], ones_col)
            gates.append(g)
        return gates[-1]

    # ---------- attention pairs ----------
    class Pair:
        def __init__(self, qc, p):
            self.qc, self.p = qc, p
            # Schraudolph only in the ACT-bound windows (not pairs (0,0)/(1,0),
            # whose windows are PE/DMA-bound and leave ACT slack)
            self.use_schr = not (p == 0)
            self.es = [None] * NG
            self.pvh = None
            self.at = attnp.tile([128, 512], BF16, tag=f"at{qc}_{p}",
                                 name=f"at{qc}_{p}")

        def scores_grp(self, g):
            for h in range(2):
                sp = psS_slot()
                for kt in range(GK):
                    j = g * GK + kt
                    nc.tensor.matmul(
                        sp[:, kt, :],
                        lhsT=kT_t[self.p][ds(64 * h, 64), ts(j, 128)],
                        rhs=qT_t[self.p][ds(64 * h, 64), ds(self.qc * 512, 512)],
                        start=True, stop=True,
                    )
                es = esp.tile([128, GK, 512], I16, tag="es",
                              name=f"es{self.qc}_{self.p}_{g}_{h}")
                if self.use_schr and g in SCHR_GRPS:
                    nc.vector.tensor_scalar(es, sp, SCHR_A, SCHR_B, OP.mult, OP.add)
                else:
                    nc.scalar.activation(es[:, :, :].bitcast(BF16), sp, AF.Exp)
                self.es[g] = self.es[g] or [None, None]
                self.es[g][h] = es

        def pv_grp(self, g):
            if self.pvh is None:
                self.pvh = [[psV.tile([HD + 1, 512], F32, tag="pv",
                                      name=f"pv{self.qc}_{self.p}_{h}_{half}")
                             for half in range(2)] for h in range(2)]
            for h in range(2):
                es = self.es[g][h]
                for kt in range(GK):
                    j = g * GK + kt
                    for half in range(2):
                        nc.tensor.matmul(
                            self.pvh[h][half],
                            lhsT=vext_t[j][ds(64 * half, 64), 2 * self.p + h, :],
                            rhs=es[ds(64 * half, 64), kt, :].bitcast(BF16),
                            start=(j == 0), stop=(j == NKT - 1),
                        )
            self.es[g] = None

        def end(self, snap_idx=None):
            gate = None
            for h in range(2):
                A, B = self.pvh[h]
                a_sb = smallp.tile([HD + 1, 512], F32, tag="asb",
                                   name=f"asb{self.qc}{self.p}{h}")
                nc.vector.tensor_copy(a_sb, A)
                af = smallp.tile([HD + 1, 512], F32, tag="af",
                                 name=f"af{self.qc}{self.p}{h}")
                nc.vector.tensor_tensor(af, a_sb, B, OP.add)
                rr = smallp.tile([1, 512], F32, tag="rr", name=f"rr{self.qc}{self.p}{h}")
                with nc.allow_low_precision("1/l feeds f32r broadcast matmul"):
                    nc.vector.reciprocal(_r(rr), af[ds(HD, 1), :])
                rbc = psV.tile([64, 512], F32, tag="pv", name=f"rbc{self.qc}{self.p}{h}")
                nc.tensor.matmul(rbc, lhsT=_r(ones64), rhs=_r(rr), start=True, stop=True)
                gate = nc.vector.tensor_mul(self.at[ds(64 * h, 64), :],
                                            af[ds(0, HD), :], rbc)
            if prof is not None and snap_idx is not None:
                prof.snap(snap_idx, gate)
            self.pvh = None

    pairs = {}
    for qc in range(2):
        for p in range(3):
            pairs[(qc, p)] = Pair(qc, p)

    def oproj_unit(qc, qs, ob):
        for oc in range(2):
            po = psS.tile([128, 2, 512], F32, tag=f"psS{slot_ctr[0] % 2}",
                          name=f"po{qc}_{qs}_{oc}")
            slot_ctr[0] += 1
            for p in range(3):
                nc.tensor.matmul(
                    po[:, 0, 0:384],
                    lhsT=pairs[(qc, p)].at[:, ts(qs, 128)],
                    rhs=wo_t[p][:, ts(oc, 384)],
                    start=(p == 0), stop=(p == 2),
                )
            nc.vector.tensor_copy(ob[:, ts(oc, 384)], po[:, 0, 0:384])
        return nc.sync.dma_start(out=OUT[ds(qc * 512 + qs * 128, 128), :], in_=ob)

    # ================= weave =================
    # Within-pair pipeline: PV lags scores by PVLAG groups; vproj rides in
    # pair (0,0)'s window, kproj p=1,2 in pair (1,0)'s.
    PVLAG = 2
    P00, P10 = pairs[(0, 0)], pairs[(1, 0)]

    # W0: qproj + kproj p=0
    qgate = None
    for p in range(3):
        qgate = qproj_unit(p)
    if prof is not None:
        prof.snap(1, qgate)
    for ck in range(4):
        kproj_unit(0, ck)

    # W1: vproj || pair (0,0)
    vgate = None
    vin = None
    for sl in range(16):
        ck = sl // 4
        if sl % 4 == 0:
            vin = [vinp.tile([128, 1024], BF16, tag=f"vin{i}", name=f"vin{i}_{ck}")
                   for i in range(6)]
            for i in range(6):
                nc.sync.dma_start(out=vin[i], in_=VT[ts(i, 128), ds(ck * 1024, 1024)])
        vgate = vproj_slot(vin, ck, sl % 4)
        P00.scores_grp(sl)
        if sl >= PVLAG:
            P00.pv_grp(sl - PVLAG)
    if prof is not None:
        prof.snap(3, vgate)
    for g in range(NG - PVLAG, NG):
        P00.pv_grp(g)
    P00.end(snap_idx=4)

    # W2: kproj p=1,2 || pair (1,0)
    kgate = None
    for u in range(8):
        p_ = 1 + u // 4
        kgate = kproj_unit(p_, u % 4)
        for g2 in (2 * u, 2 * u + 1):
            P10.scores_grp(g2)
            if g2 >= PVLAG:
                P10.pv_grp(g2 - PVLAG)
    if prof is not None:
        prof.snap(2, kgate)
    for g in range(NG - PVLAG, NG):
        P10.pv_grp(g)
    P10.end(snap_idx=5)

    # W3..W6: pure pairs
    seq = [(0, 1), (1, 1), (0, 2), (1, 2)]
    for idx, cur in enumerate(seq):
        cp = pairs[cur]
        for g in range(NG):
            cp.scores_grp(g)
            if g >= PVLAG:
                cp.pv_grp(g - PVLAG)
            if cur == (1, 2) and g % 4 == 3:
                qs = g // 4
                ob = obp.tile([128, D_MODEL], F32, tag="ob", name=f"ob0_{qs}")
                d = oproj_unit(0, qs, ob)
                if prof is not None and qs == 3:
                    prof.snap(10, d)
        for g in range(NG - PVLAG, NG):
            cp.pv_grp(g)
        cp.end(snap_idx=6 + idx)
    for qs in range(4):
        ob = obp.tile([128, D_MODEL], F32, tag="ob", name=f"ob1_{qs}")
        d = oproj_unit(1, qs, ob)
        if prof is not None and qs == 3:
            prof.snap(11, d)

    for pool in [vinp, psV, psS, smallp, obp, attnp, esp, persist, consts]:
        pool.release()


_nc_cache = {}


def build_nc(schr16=0, split_waits=True, timing_mode=False, profile_ladder=False):
    key = (schr16, split_waits, timing_mode, profile_ladder)
    if key in _nc_cache:
        return _nc_cache[key]
    nc = bass.Bass()
    if timing_mode:
        nc.declare_dram_parameter("DUMMY", [1, 128], F32, isOutput=False)
        io = {
            "QT": nc.dram_tensor("QT", [D_MODEL, QR], BF16),
            "KT": nc.dram_tensor("KT", [D_MODEL, S], BF16),
            "VT": nc.dram_tensor("VT", [D_MODEL, S], BF16),
            "WQ": nc.dram_tensor("WQ", [D_MODEL, C], BF16),
            "WK": nc.dram_tensor("WK", [D_MODEL, C], BF16),
            "WV": nc.dram_tensor("WV", [D_MODEL, C], BF16),
            "WO": nc.dram_tensor("WO", [C, D_MODEL], BF16),
            "BQ": nc.dram_tensor("BQ", [C], F32),
            "OUT": nc.declare_dram_parameter("OUT", [QR, D_MODEL], F32, isOutput=True),
        }
    else:
        io = {
            "QT": nc.declare_dram_parameter("QT", [D_MODEL, QR], BF16, isOutput=False),
            "KT": nc.declare_dram_parameter("KT", [D_MODEL, S], BF16, isOutput=False),
            "VT": nc.declare_dram_parameter("VT", [D_MODEL, S], BF16, isOutput=False),
            "WQ": nc.declare_dram_parameter("WQ", [D_MODEL, C], BF16, isOutput=False),
            "WK": nc.declare_dram_parameter("WK", [D_MODEL, C], BF16, isOutput=False),
            "WV": nc.declare_dram_parameter("WV", [D_MODEL, C], BF16, isOutput=False),
            "WO": nc.declare_dram_parameter("WO", [C, D_MODEL], BF16, isOutput=False),
            "BQ": nc.declare_dram_parameter("BQ", [C], F32, isOutput=False),
            "OUT": nc.declare_dram_parameter("OUT", [QR, D_MODEL], F32, isOutput=True),
        }
    prof = None
    prog_ap = None
    if profile_ladder:
        PROG = nc.declare_dram_parameter(
            "PROG", [PROF_NSNAP, PROF_LK], mybir.dt.int32, isOutput=True)
        prog_ap = nc.alloc_sbuf_tensor("prog_ticks", [1, PROF_LK], mybir.dt.int32).ap()
        prof = _Prof(nc, prog_ap, PROG)
    with TileContext(nc) as tc:
        _emit_body(nc, tc, io, schr16=schr16, prof=prof)
    if profile_ladder:
        _emit_prof_ladder(nc, prog_ap)
    if split_waits:
        _split_excess_waits(nc)
    _nc_cache[key] = nc
    return nc


def make_in_maps(Q, K, V, Wq, bq, Wk, bk, Wv, bv, Wo, bo):
    import ml_dtypes
    BF = ml_dtypes.bfloat16
    Qm = np.asarray(Q, np.float32).reshape(S, D_MODEL)
    Km = np.asarray(K, np.float32).reshape(S, D_MODEL)
    Vm = np.asarray(V, np.float32).reshape(S, D_MODEL)
    QTf = np.ascontiguousarray(Qm.T).astype(BF)
    KTf = np.ascontiguousarray(Km.T).astype(BF)
    VTf = np.ascontiguousarray(Vm.T).astype(BF)
    Wq = np.asarray(Wq, np.float32); Wk = np.asarray(Wk, np.float32)
    Wv = np.asarray(Wv, np.float32); Wo = np.asarray(Wo, np.float32)
    bq = np.asarray(bq, np.float32); bv = np.asarray(bv, np.float32)
    bo = np.asarray(bo, np.float32)

    in_maps = []
    for c in range(NCORES):
        g, b = divmod(c, QB)
        ch = slice(g * C, (g + 1) * C)
        in_maps.append({
            "QT": np.ascontiguousarray(QTf[:, b * QR : (b + 1) * QR]),
            "KT": KTf,
            "VT": VTf,
            "WQ": np.ascontiguousarray(Wq[:, ch]).astype(BF),
            "WK": np.ascontiguousarray(Wk[:, ch]).astype(BF),
            "WV": np.ascontiguousarray(Wv[:, ch]).astype(BF),
            "WO": np.ascontiguousarray(Wo[ch, :]).astype(BF),
            "BQ": np.ascontiguousarray(bq[ch] * np.float32(SCALE)),
        })
    host_const = (bv @ Wo + bo).astype(np.float32)
    return in_maps, host_const


def kernel(Q, K, V, Wq, bq, Wk, bk, Wv, bv, Wo, bo, schr16=0):
    nc = build_nc(schr16=schr16)
    in_maps, host_const = make_in_maps(Q, K, V, Wq, bq, Wk, bk, Wv, bv, Wo, bo)
    res = run_bass_kernel_spmd(nc, in_maps, core_ids=list(range(NCORES)))
    out = np.zeros((S, D_MODEL), np.float32)
    for c in range(NCORES):
        g, b = divmod(c, QB)
        out[b * QR : (b + 1) * QR, :] += res.results[c]["OUT"]
    out += host_const[None, :]
    return out.reshape(1, S, D_MODEL)


# revision 3
# speedup vs baseline: 1.3294x; 1.0588x over previous
"""Trainium2 Bass kernel v2 for nn_MultiHeadAttention (B=1, S=4096, d=768, 12 heads).

Sharding (8 cores): 2 head-groups (6 heads / 384 ch) x 4 query-blocks (1024 rows).
All-bf16 data path (f32 PSUM accum), f32r only for tiny broadcast matmuls.

Design (per core), driven by ACT (softmax exp) saturation -- exp of 25.2M
scores/core through the 128-lane 1.2GHz ScalarE is the hard floor (~35us per
512q x 128ch pair):
  - pairs (qc, p) of 512 queries x 128 channels: (0,0),(1,0) ride the
    projection windows, then (0,1),(0,2),(1,1),(1,2)
  - per pair, 16 groups of GK=2 key-tiles, software-pipelined within the pair:
    scores (4 MMs, the 2 heads row-group concurrent on the PE) -> exp (one
    N=1024 ACT call per head, PSUM-direct) -> PV 2 groups behind (65-col
    serial MMs; ones-row gives the softmax denominator)
  - PSUM: psS h0/h1 scores slots (4 banks, exp-paced ring); psV: pvh tag
    (2 banks, PV accumulators + 1/l broadcast) + pp tag (2 banks, a dedicated
    projection ring so projections never serialize behind exp)
  - projections: weights + full KT resident in SBUF, kproj p-major so pair
    (0,0) starts right after kproj p=0; vproj rides pair (0,0)'s window,
    kproj p=1,2 ride pair (1,0)'s; oproj(qc=0) hides under pair (1,1).
Host: sums the 2 head-group partials per query block and adds bv@Wo + bo.
"""

import sys

sys.path.insert(0, "/opt/trn_rl_repo")

import numpy as np

import concourse.bass as bass
import concourse.mybir as mybir
from concourse.bass import ts, ds
from concourse.bass_utils import run_bass_kernel_spmd
from concourse.tile import TileContext

D_MODEL = 768
S = 4096
NH = 12
HD = 64
HG = 2
QB = 4
C = D_MODEL // HG       # 384 channels per head-group
NHL = NH // HG          # 6 heads per group
QR = S // QB            # 1024 query rows per block
NCORES = 8
SCALE = float(1.0 / np.sqrt(np.float32(D_MODEL)))
NKT = S // 128          # 32 key tiles
GK = 2                  # key tiles per group
NG = NKT // GK          # 16 groups per pair

F32 = mybir.dt.float32
F32R = mybir.dt.float32r
BF16 = mybir.dt.bfloat16
I16 = mybir.dt.int16
AF = mybir.ActivationFunctionType
OP = mybir.AluOpType

# Schraudolph exp in bf16 bits: bits = s * (2^7/ln2) + SCHR_B  (i16, bitcast bf16)
SCHR_A = 184.6650390625
SCHR_B = 16250.4            # HW-calibrated: centers the sawtooth error at +-3.3%


def _r(ap):
    return ap.bitcast(F32R)


def _split_excess_waits(nc, max_waits=1):
    """walrus rejects instructions carrying more than one semaphore wait."""
    n_split = 0
    for f in nc.m.functions:
        for blk in f.blocks:
            new_insts = []
            for inst in blk.instructions:
                si = inst.sync_info
                if si is not None and si.on_wait and len(si.on_wait) > max_waits:
                    waits = list(si.on_wait)
                    keep = waits[-max_waits:]
                    extra = waits[:-max_waits]
                    for i in range(0, len(extra), max_waits):
                        chunk = extra[i : i + max_waits]
                        nop = mybir.InstNoOp(
                            name=f"{inst.name}_wsplit_{i}",
                            ins=[],
                            outs=[],
                            engine=inst.engine,
                            sync_info=mybir.SyncInfo(on_wait=chunk, on_update=[]),
                        )
                        new_insts.append(nop)
                        n_split += 1
                    si.on_wait = keep
                new_insts.append(inst)
            blk.instructions = new_insts
    return n_split


PROF_LK = 256
PROF_TICK_CYC = 4800
PROF_NSNAP = 12


class _Prof:
    def __init__(self, nc, prog_ap, PROG):
        self.nc = nc
        self.prog_ap = prog_ap
        self.PROG = PROG

    def snap(self, idx, gate):
        from concourse.tile_rust import add_dep_helper
        d = self.nc.sync.dma_start(out=self.PROG[ds(idx, 1), :], in_=self.prog_ap)
        add_dep_helper(d.ins, gate.ins, sync=True, reason=f"prof snap {idx}")


def _emit_prof_ladder(nc, prog_ap):
    ladder = []
    reg_ctx = nc.gpsimd.register("prof_tick")
    reg = reg_ctx.__enter__()
    z = nc.gpsimd.reg_alu(reg, 0, 0, OP.add)
    ladder.append(z.ins)
    for i in range(PROF_LK):
        s = nc.gpsimd.store(prog_ap[0:1, ds(i, 1)], reg)
        ladder.append(s.ins)
    for i in range(PROF_LK):
        a = nc.gpsimd.reg_alu(reg, reg, 1, OP.add)
        ladder.append(a.ins)
        s = nc.gpsimd.store(prog_ap[0:1, ds(i, 1)], reg)
        ladder.append(s.ins)
        n = nc.gpsimd.nop(cycle_cnt=PROF_TICK_CYC, nofuse=True)
        ladder.append(n.ins)
    ladder_set = set(id(x) for x in ladder)
    f = nc.m.functions[0]
    for blk in f.blocks:
        blk.instructions = [x for x in blk.instructions if id(x) not in ladder_set]
    for blk in f.blocks:
        if blk.name.startswith("tile_context"):
            blk.instructions[0:0] = ladder
            return
    raise RuntimeError("profiler: no tile_context block found for tick ladder")


def _emit_body(nc, tc, io, schr16=0, prof=None, dbg=False, pvlag=2, esbufs=6):
    QT, KT, VT, WQ, WK, WV, WO, BQ, OUT = (
        io["QT"], io["KT"], io["VT"], io["WQ"], io["WK"], io["WV"], io["WO"],
        io["BQ"], io["OUT"],
    )
    SCHR_GRPS = {
        0: set(), 1: {8}, 2: {5, 11}, 3: {4, 9, 14}, 4: {3, 7, 11, 15},
        5: {2, 5, 8, 11, 14}, 6: {1, 4, 7, 10, 13, 15}, 8: {1, 3, 5, 7, 9, 11, 13, 15},
    }[schr16]

    consts = tc.alloc_tile_pool(name="consts", bufs=1)
    persist = tc.alloc_tile_pool(name="persist", bufs=1)
    esp = tc.alloc_tile_pool(name="esp", bufs=esbufs)
    attnp = tc.alloc_tile_pool(name="attnp", bufs=1)
    obp = tc.alloc_tile_pool(name="obp", bufs=2)
    smallp = tc.alloc_tile_pool(name="smallp", bufs=2)
    psS = tc.alloc_tile_pool(name="psS", bufs=1, space="PSUM")
    psV = tc.alloc_tile_pool(name="psV", bufs=2, space="PSUM")

    # ---- weights -> SBUF (bf16) ----
    wq_t, wk_t, wv_t = [], [], []
    for i in range(6):
        wq = consts.tile([128, C], BF16, tag=f"wq{i}", name=f"wq{i}")
        d0 = nc.sync.dma_start(out=wq, in_=WQ[ts(i, 128), :])
        if prof is not None and i == 0 and not dbg:
            prof.snap(0, d0)
        wq_t.append(wq)
    for i in range(6):
        wk = consts.tile([128, C], BF16, tag=f"wk{i}", name=f"wk{i}")
        nc.sync.dma_start(out=wk, in_=WK[ts(i, 128), :])
        wk_t.append(wk)
    for i in range(6):
        wv = consts.tile([128, C], BF16, tag=f"wv{i}", name=f"wv{i}")
        nc.sync.dma_start(out=wv, in_=WV[ts(i, 128), :])
        wv_t.append(wv)
    wo_t = []
    for p in range(3):
        wo = consts.tile([128, D_MODEL], BF16, tag=f"wo{p}", name=f"wo{p}")
        nc.sync.dma_start(out=wo, in_=WO[ts(p, 128), :])
        wo_t.append(wo)
    bq_t = []
    for p in range(3):
        bq = consts.tile([128, 1], F32, tag=f"bq{p}", name=f"bq{p}")
        nc.sync.dma_start(out=bq, in_=BQ[ts(p, 128)].rearrange("(p one) -> p one", one=1))
        bq_t.append(bq)
    ones64 = consts.tile([1, 64], F32, tag="ones64", name="ones64")
    nc.vector.memset(ones64, 1.0)
    ones_col = consts.tile([128, NHL], F32, tag="ones_col", name="ones_col")
    nc.vector.memset(ones_col, 1.0)

    # ---- persistent activations ----
    kT_t = [persist.tile([128, S], BF16, tag=f"kT{p}", name=f"kT{p}") for p in range(3)]
    qT_t = [persist.tile([128, QR], BF16, tag=f"qT{p}", name=f"qT{p}") for p in range(3)]
    vext_t = [persist.tile([128, NHL, HD + 1], BF16, tag=f"vx{j}", name=f"vx{j}")
              for j in range(NKT)]
    # inputs resident
    ktin = [persist.tile([128, S], BF16, tag=f"ktin{i}", name=f"ktin{i}") for i in range(6)]
    qin = [persist.tile([128, QR], BF16, tag=f"qin{i}", name=f"qin{i}") for i in range(6)]
    for i in range(6):
        nc.sync.dma_start(out=qin[i], in_=QT[ts(i, 128), :])
    for i in range(6):
        nc.sync.dma_start(out=ktin[i], in_=KT[ts(i, 128), :])
    vinp = tc.alloc_tile_pool(name="vinp", bufs=2)

    # psS slot allocator: alternate h0/h1 slots for projection chunks too
    slot_ctr = [0]

    def psS_slot():
        t = psS.tile([128, 2, 512], F32, tag=f"psS{slot_ctr[0] % 2}",
                     name=f"slot{slot_ctr[0]}")
        slot_ctr[0] += 1
        return t

    # ---------- projection units (psV "pv" ring transient slots) ----------
    def qproj_unit(p, half):
        sp = psV.tile([128, 512], F32, tag="pp", name=f"qp{p}_{half}")
        for i in range(6):
            nc.tensor.matmul(
                sp, lhsT=wq_t[i][:, ts(p, 128)],
                rhs=qin[i][:, ds(half * 512, 512)],
                start=(i == 0), stop=(i == 5),
            )
        return nc.vector.tensor_scalar(
            qT_t[p][:, ds(half * 512, 512)], sp, SCALE, bq_t[p],
            OP.mult, OP.add)

    def kproj_unit(p, c8):
        sp = psV.tile([128, 512], F32, tag="pp", name=f"kp{p}_{c8}")
        for i in range(6):
            nc.tensor.matmul(
                sp, lhsT=wk_t[i][:, ts(p, 128)],
                rhs=ktin[i][:, ds(c8 * 512, 512)],
                start=(i == 0), stop=(i == 5),
            )
        return nc.vector.tensor_copy(kT_t[p][:, ds(c8 * 512, 512)], sp)

    def vproj_unit(vin, ck, jj):
        # key tile j = ck*8 + jj
        j = ck * 8 + jj
        sp = psV.tile([128, 512], F32, tag="pp", name=f"vp{j}")
        for i in range(6):
            nc.tensor.matmul(
                sp[:, 0:C], lhsT=vin[i][:, ds(jj * 128, 128)],
                rhs=wv_t[i], start=(i == 0), stop=(i == 5),
            )
        nc.vector.tensor_copy(
            vext_t[j][:, :, 0:HD], sp[:, 0:C].rearrange("p (h d) -> p h d", h=NHL))
        return nc.vector.tensor_copy(vext_t[j][:, :, HD], ones_col)

    # ---------- attention pairs ----------
    class Pair:
        def __init__(self, qc, p):
            self.qc, self.p = qc, p
            # Schraudolph only in the ACT-bound windows (not pairs (0,0)/(1,0),
            # whose windows are PE/DMA-bound and leave ACT slack)
            self.use_schr = not (p == 0)
            self.es = [None] * NG
            self.pvh = None
            self.at = attnp.tile([128, 512], BF16, tag=f"at{qc}_{p}",
                                 name=f"at{qc}_{p}")

        def scores_grp(self, g):
            for h in range(2):
                sp = psS_slot()
                for kt in range(GK):
                    j = g * GK + kt
                    nc.tensor.matmul(
                        sp[:, kt, :],
                        lhsT=kT_t[self.p][ds(64 * h, 64), ts(j, 128)],
                        rhs=qT_t[self.p][ds(64 * h, 64), ds(self.qc * 512, 512)],
                        start=True, stop=True,
                    )
                es = esp.tile([128, GK, 512], I16, tag="es",
                              name=f"es{self.qc}_{self.p}_{g}_{h}")
                if self.use_schr and g in SCHR_GRPS:
                    ei = nc.vector.tensor_scalar(es, sp, SCHR_A, SCHR_B, OP.mult, OP.add)
                else:
                    ei = nc.scalar.activation(es[:, :, :].bitcast(BF16), sp, AF.Exp)
                if dbg and prof is not None and (self.qc, self.p) == (0, 2) \
                        and h == 1 and g % 4 == 3:
                    prof.snap(g // 4, ei)
                self.es[g] = self.es[g] or [None, None]
                self.es[g][h] = es

        def pv_grp(self, g):
            # pairs riding the projection windows use serial 65-col PV (2
            # accumulators, pp slots busy with projections); pure pairs split
            # the contraction in rg-halves (v2 scheme) with the B-half
            # accumulators in the then-idle pp slots.
            split = self.p != 0
            if self.pvh is None:
                if split:
                    self.pvh = [[psV.tile([HD + 1, 512], F32, tag=tag,
                                          name=f"pv{self.qc}_{self.p}_{h}_{tag}")
                                 for tag in ("pvh", "pp")] for h in range(2)]
                else:
                    self.pvh = [psV.tile([HD + 1, 512], F32, tag="pvh",
                                         name=f"pv{self.qc}_{self.p}_{h}")
                                for h in range(2)]
            for h in range(2):
                es = self.es[g][h]
                head = 2 * self.p + h
                for kt in range(GK):
                    j = g * GK + kt
                    if split:
                        for half in range(2):
                            nc.tensor.matmul(
                                self.pvh[h][half],
                                lhsT=vext_t[j][ds(64 * half, 64), head, :],
                                rhs=es[ds(64 * half, 64), kt, :].bitcast(BF16),
                                start=(j == 0), stop=(j == NKT - 1),
                            )
                    else:
                        nc.tensor.matmul(
                            self.pvh[h],
                            lhsT=vext_t[j][:, head, :],
                            rhs=es[:, kt, :].bitcast(BF16),
                            start=(j == 0), stop=(j == NKT - 1),
                        )
            self.es[g] = None

        def end(self, snap_idx=None):
            gate = None
            split = self.p != 0
            for h in range(2):
                if split:
                    A, B = self.pvh[h]
                else:
                    A, B = self.pvh[h], None
                a_sb = smallp.tile([HD + 1, 512], F32, tag="asb",
                                   name=f"asb{self.qc}{self.p}{h}", bufs=2)
                nc.vector.tensor_copy(a_sb, A)
                if B is not None:
                    af = smallp.tile([HD + 1, 512], F32, tag="af",
                                     name=f"af{self.qc}{self.p}{h}", bufs=2)
                    nc.vector.tensor_tensor(af, a_sb, B, OP.add)
                else:
                    af = a_sb
                rr = smallp.tile([1, 512], F32, tag="rr", name=f"rr{self.qc}{self.p}{h}")
                with nc.allow_low_precision("1/l feeds f32r broadcast matmul"):
                    nc.vector.reciprocal(_r(rr), af[ds(HD, 1), :])
                rbc = psV.tile([64, 512], F32, tag="pvh", name=f"rbc{self.qc}{self.p}{h}")
                nc.tensor.matmul(rbc, lhsT=_r(ones64), rhs=_r(rr), start=True, stop=True)
                gate = nc.vector.tensor_mul(self.at[ds(64 * h, 64), :],
                                            af[ds(0, HD), :], rbc)
            if prof is not None and snap_idx is not None:
                prof.snap(snap_idx, gate)
            self.pvh = None

    pairs = {}
    for qc in range(2):
        for p in range(3):
            pairs[(qc, p)] = Pair(qc, p)

    def oproj_unit(qc, qs, ob):
        for oc in range(2):
            po = psS.tile([128, 2, 512], F32, tag=f"psS{slot_ctr[0] % 2}",
                          name=f"po{qc}_{qs}_{oc}")
            slot_ctr[0] += 1
            for p in range(3):
                nc.tensor.matmul(
                    po[:, 0, 0:384],
                    lhsT=pairs[(qc, p)].at[:, ts(qs, 128)],
                    rhs=wo_t[p][:, ts(oc, 384)],
                    start=(p == 0), stop=(p == 2),
                )
            nc.vector.tensor_copy(ob[:, ts(oc, 384)], po[:, 0, 0:384])
        return nc.sync.dma_start(out=OUT[ds(qc * 512 + qs * 128, 128), :], in_=ob)

    # ================= weave =================
    # Within-pair pipeline: PV lags scores by PVLAG groups; vproj rides in
    # pair (0,0)'s window, kproj p=1,2 in pair (1,0)'s.
    PVLAG = pvlag
    P00, P10 = pairs[(0, 0)], pairs[(1, 0)]

    # W0: qproj + kproj p=0
    qgate = None
    for p in range(3):
        for half in range(2):
            qgate = qproj_unit(p, half)
    if prof is not None and not dbg:
        prof.snap(1, qgate)
    for c8 in range(8):
        kproj_unit(0, c8)

    # W1: vproj || pair (0,0)
    vgate = None
    vin = None
    for j in range(NKT):
        ck = j // 8
        if j % 8 == 0:
            vin = [vinp.tile([128, 1024], BF16, tag=f"vin{i}", name=f"vin{i}_{ck}")
                   for i in range(6)]
            for i in range(6):
                nc.sync.dma_start(out=vin[i], in_=VT[ts(i, 128), ds(ck * 1024, 1024)])
        vgate = vproj_unit(vin, ck, j % 8)
        if j % 2 == 1:
            g = j // 2
            P00.scores_grp(g)
            if g >= PVLAG:
                P00.pv_grp(g - PVLAG)
    if prof is not None and not dbg:
        prof.snap(3, vgate)
    for g in range(NG - PVLAG, NG):
        P00.pv_grp(g)
    P00.end(snap_idx=4)

    # W2: kproj p=1,2 || pair (1,0)
    kgate = None
    for u in range(16):
        p_ = 1 + u // 8
        kgate = kproj_unit(p_, u % 8)
        g2 = u
        P10.scores_grp(g2)
        if g2 >= PVLAG:
            P10.pv_grp(g2 - PVLAG)
    if prof is not None and not dbg:
        prof.snap(2, kgate)
    for g in range(NG - PVLAG, NG):
        P10.pv_grp(g)
    P10.end(snap_idx=5)

    # W3..W6: pure pairs ((0,*) first so oproj(0) can hide under (1,1))
    seq = [(0, 1), (0, 2), (1, 1), (1, 2)]
    for idx, cur in enumerate(seq):
        cp = pairs[cur]
        for g in range(NG):
            cp.scores_grp(g)
            if g >= PVLAG:
                cp.pv_grp(g - PVLAG)
            if cur == (1, 1) and g % 4 == 3:
                qs = g // 4
                ob = obp.tile([128, D_MODEL], F32, tag="ob", name=f"ob0_{qs}")
                d = oproj_unit(0, qs, ob)
                if prof is not None and qs == 3 and not dbg:
                    prof.snap(10, d)
        for g in range(NG - PVLAG, NG):
            cp.pv_grp(g)
        cp.end(snap_idx=6 + idx)
    for qs in range(4):
        ob = obp.tile([128, D_MODEL], F32, tag="ob", name=f"ob1_{qs}")
        d = oproj_unit(1, qs, ob)
        if prof is not None and qs == 3 and not dbg:
            prof.snap(11, d)

    for pool in [vinp, psV, psS, smallp, obp, attnp, esp, persist, consts]:
        pool.release()


_nc_cache = {}


def build_nc(schr16=0, split_waits=True, timing_mode=False, profile_ladder=False,
             dbg=False, pvlag=2, esbufs=6):
    key = (schr16, split_waits, timing_mode, profile_ladder, dbg, pvlag, esbufs)
    if key in _nc_cache:
        return _nc_cache[key]
    nc = bass.Bass()
    if timing_mode:
        nc.declare_dram_parameter("DUMMY", [1, 128], F32, isOutput=False)
        io = {
            "QT": nc.dram_tensor("QT", [D_MODEL, QR], BF16),
            "KT": nc.dram_tensor("KT", [D_MODEL, S], BF16),
            "VT": nc.dram_tensor("VT", [D_MODEL, S], BF16),
            "WQ": nc.dram_tensor("WQ", [D_MODEL, C], BF16),
            "WK": nc.dram_tensor("WK", [D_MODEL, C], BF16),
            "WV": nc.dram_tensor("WV", [D_MODEL, C], BF16),
            "WO": nc.dram_tensor("WO", [C, D_MODEL], BF16),
            "BQ": nc.dram_tensor("BQ", [C], F32),
            "OUT": nc.declare_dram_parameter("OUT", [QR, D_MODEL], F32, isOutput=True),
        }
    else:
        io = {
            "QT": nc.declare_dram_parameter("QT", [D_MODEL, QR], BF16, isOutput=False),
            "KT": nc.declare_dram_parameter("KT", [D_MODEL, S], BF16, isOutput=False),
            "VT": nc.declare_dram_parameter("VT", [D_MODEL, S], BF16, isOutput=False),
            "WQ": nc.declare_dram_parameter("WQ", [D_MODEL, C], BF16, isOutput=False),
            "WK": nc.declare_dram_parameter("WK", [D_MODEL, C], BF16, isOutput=False),
            "WV": nc.declare_dram_parameter("WV", [D_MODEL, C], BF16, isOutput=False),
            "WO": nc.declare_dram_parameter("WO", [C, D_MODEL], BF16, isOutput=False),
            "BQ": nc.declare_dram_parameter("BQ", [C], F32, isOutput=False),
            "OUT": nc.declare_dram_parameter("OUT", [QR, D_MODEL], F32, isOutput=True),
        }
    prof = None
    prog_ap = None
    if profile_ladder:
        PROG = nc.declare_dram_parameter(
            "PROG", [PROF_NSNAP, PROF_LK], mybir.dt.int32, isOutput=True)
        prog_ap = nc.alloc_sbuf_tensor("prog_ticks", [1, PROF_LK], mybir.dt.int32).ap()
        prof = _Prof(nc, prog_ap, PROG)
    with TileContext(nc) as tc:
        _emit_body(nc, tc, io, schr16=schr16, prof=prof, dbg=dbg, pvlag=pvlag,
                   esbufs=esbufs)
    if profile_ladder:
        _emit_prof_ladder(nc, prog_ap)
    if split_waits:
        _split_excess_waits(nc)
    _nc_cache[key] = nc
    return nc


def make_in_maps(Q, K, V, Wq, bq, Wk, bk, Wv, bv, Wo, bo):
    import ml_dtypes
    BF = ml_dtypes.bfloat16
    Qm = np.asarray(Q, np.float32).reshape(S, D_MODEL)
    Km = np.asarray(K, np.float32).reshape(S, D_MODEL)
    Vm = np.asarray(V, np.float32).reshape(S, D_MODEL)
    QTf = np.ascontiguousarray(Qm.T).astype(BF)
    KTf = np.ascontiguousarray(Km.T).astype(BF)
    VTf = np.ascontiguousarray(Vm.T).astype(BF)
    Wq = np.asarray(Wq, np.float32); Wk = np.asarray(Wk, np.float32)
    Wv = np.asarray(Wv, np.float32); Wo = np.asarray(Wo, np.float32)
    bq = np.asarray(bq, np.float32); bv = np.asarray(bv, np.float32)
    bo = np.asarray(bo, np.float32)

    in_maps = []
    for c in range(NCORES):
        g, b = divmod(c, QB)
        ch = slice(g * C, (g + 1) * C)
        in_maps.append({
            "QT": np.ascontiguousarray(QTf[:, b * QR : (b + 1) * QR]),
            "KT": KTf,
            "VT": VTf,
            "WQ": np.ascontiguousarray(Wq[:, ch]).astype(BF),
            "WK": np.ascontiguousarray(Wk[:, ch]).astype(BF),
            "WV": np.ascontiguousarray(Wv[:, ch]).astype(BF),
            "WO": np.ascontiguousarray(Wo[ch, :]).astype(BF),
            "BQ": np.ascontiguousarray(bq[ch] * np.float32(SCALE)),
        })
    host_const = (bv @ Wo + bo).astype(np.float32)
    return in_maps, host_const


def kernel(Q, K, V, Wq, bq, Wk, bk, Wv, bv, Wo, bo, schr16=0):
    nc = build_nc(schr16=schr16)
    in_maps, host_const = make_in_maps(Q, K, V, Wq, bq, Wk, bk, Wv, bv, Wo, bo)
    res = run_bass_kernel_spmd(nc, in_maps, core_ids=list(range(NCORES)))
    out = np.zeros((S, D_MODEL), np.float32)
    for c in range(NCORES):
        g, b = divmod(c, QB)
        out[b * QR : (b + 1) * QR, :] += res.results[c]["OUT"]
    out += host_const[None, :]
    return out.reshape(1, S, D_MODEL)
